# revision 25
# baseline (speedup 1.0000x reference)
"""Trainium2 Bass kernel for BasicDMPNN (gnn_message_passing).

Strategy:
  - Nodes are partitioned contiguously across 8 cores (12500 each); every
    edge is owned by the core that owns its dst node, so the segment-sum
    (dma_scatter_add) is core-local.
  - The edge MLPs fold into tiny tables:
      msg_0[e]   = relu(Ci[code[e]])
      msg_r[e]   = relu(Cu[code[e]] + (agg_{r-1} @ Wu2)[src[e]])
    with code[e] = 4*x[src[e]] + edge_attr[e] (476 entries), because
    ab = [atom_table[x[src]], bond_table[ea]] enters the MLPs linearly.
  - Per round: each core computes aggW = agg @ Wu2 for its node slice
    (PE), AllGather makes the full table visible to all cores, then the
    edge pass is dma_gather(aggW)[src] + base + relu + dma_scatter_add
    by dst.
  - dma_gather / dma_scatter_add are limited to ~1024 indices per call
    (SWDGE ring) and scatter-add loses updates for duplicate indices
    within one call; calls targeting the same tensor serialize (WAW) and
    are then exact. So blocks are 1024 edges, each dst appears at most
    once per block (host round-robin), and blocks rotate over 4
    accumulator tensors (summed at the end of the round) to keep four
    independent WAW chains.
  - base[e] = Cu[code[e]] is materialized once (round 0, bf16) and
    streamed sequentially in rounds 1-4.
  - Molecule readout: one-hot matmul (host-built selection matrices) into
    a per-core 1024-mol window held in PSUM, AllGather of the transposed
    partials, combine at static per-core offsets, then the MLP head.
"""

import os

import numpy as np

import concourse.bacc as bacc
import concourse.bass as bass
import concourse.mybir as mybir
import concourse.tile as tile
from concourse import bass_utils
from concourse.masks import make_identity

N_CORES = 8
N_NODES = 100000
NPC = 12500          # nodes per core
NPCP = 12544         # padded node slice (98 * 128); rows >= 12500 are scratch
N_CHUNKS = 4         # src chunks so gather indices fit int16
CHUNK_ROWS = 2 * NPCP  # 25088 rows per chunk of the allgathered table
FULL_ROWS = N_CORES * NPCP  # 100352
MOLS = 2048
MOLW = 1024          # per-core mol window (each core's mols span < 512)
MSG = 64
BLOCK = 1024
TB = BLOCK // 128    # 8
NACC = 8             # parallel scatter accumulator tensors
NT = NPCP // 128     # 98 node tiles per core
ROUNDS = int(os.environ.get("DMPNN_ROUNDS", "4"))
F32 = mybir.dt.float32
BF16 = mybir.dt.bfloat16
I16 = mybir.dt.int16

_CACHE = {}


def _wrap16(idx, nblocks):
    """[nblocks*B] int -> [nblocks, 128, B//16] int16: index i of a block
    sits at [i % 16, i // 16], replicated across the 8 Q7 core groups."""
    b = idx.size // nblocks
    v = np.transpose(idx.reshape(nblocks, b // 16, 16), (0, 2, 1)).astype(np.int16)
    return np.tile(v, (1, 8, 1))


def _prep(inputs):
    x = np.asarray(inputs["x"]).astype(np.int64)
    ea = np.asarray(inputs["edge_attr"]).astype(np.int64)
    ei = np.asarray(inputs["edge_index"]).astype(np.int64)
    batch = np.asarray(inputs["batch"]).astype(np.int64)
    atom_table = np.asarray(inputs["atom_table"], np.float32)
    bond_table = np.asarray(inputs["bond_table"], np.float32)
    Wi = np.asarray(inputs["Wi"], np.float32)
    bi = np.asarray(inputs["bi"], np.float32)
    Wu = np.asarray(inputs["Wu"], np.float32)
    bu = np.asarray(inputs["bu"], np.float32)

    src, dst = ei[0], ei[1]
    a_i = atom_table @ Wi[:64]
    b_i = bond_table @ Wi[64:80]
    a_u = atom_table @ Wu[:64]
    b_u = bond_table @ Wu[64:80]
    Ci = (a_i[:, None, :] + b_i[None, :, :] + bi).reshape(476, 64)
    Cu = (a_u[:, None, :] + b_u[None, :, :] + bu).reshape(476, 64)
    cc = np.zeros((477, 128), np.float32)
    cc[:476, :64] = Ci
    cc[:476, 64:] = Cu

    code = 4 * x[src] + ea
    owner = dst // NPC
    chunk = src // (2 * NPC)
    grow = (src // NPC) * NPCP + (src % NPC)

    per_core = [[None] * N_CHUNKS for _ in range(N_CORES)]
    cnt = np.zeros((N_CORES, N_CHUNKS), np.int64)
    maxdeg = np.zeros((N_CORES, N_CHUNKS), np.int64)
    for c in range(N_CORES):
        m = owner == c
        ch = chunk[m]
        for g in range(N_CHUNKS):
            sel = np.nonzero(m)[0][ch == g]
            per_core[c][g] = sel
            cnt[c, g] = len(sel)
            if len(sel):
                maxdeg[c, g] = np.bincount(dst[sel] - c * NPC).max()
    nb = [
        int(max(np.ceil(cnt[:, g].max() * 1.03 / BLOCK), maxdeg[:, g].max()))
        for g in range(N_CHUNKS)
    ]
    nbtot = sum(nb)

    # packed per-block indices: cols 0:64 src, 64:128 dst, 128:192 code
    idx_all = np.zeros((N_CORES, nbtot, 128, 192), np.int16)
    for c in range(N_CORES):
        b0 = 0
        for g in range(N_CHUNKS):
            sel = per_core[c][g]
            nbg = nb[g]
            n = nbg * BLOCK
            s16 = np.full(n, 0, np.int64)
            d16 = np.full(n, NPCP - 1, np.int64)   # pad -> scratch row
            c16 = np.full(n, 476, np.int64)        # pad -> zero table row
            if len(sel):
                dl = dst[sel] - c * NPC
                order = np.argsort(dl, kind="stable")
                sel = sel[order]
                dl = dl[order]
                deg = np.bincount(dl, minlength=NPC)
                first = np.zeros(len(dl), bool)
                first[0] = True
                first[1:] = dl[1:] != dl[:-1]
                rank = np.arange(len(dl)) - np.repeat(
                    np.nonzero(first)[0], deg[deg > 0]
                )
                start = np.cumsum(deg) - deg
                blk = (start[dl] + rank) % nbg
                bo = np.argsort(blk, kind="stable")
                fills = np.bincount(blk, minlength=nbg)
                assert fills.max() <= BLOCK, (c, g, fills.max())
                pos = np.concatenate(
                    [b * BLOCK + np.arange(f) for b, f in enumerate(fills)]
                )
                s16[pos] = grow[sel[bo]] - g * CHUNK_ROWS
                d16[pos] = dl[bo]
                c16[pos] = code[sel[bo]]
            idx_all[c, b0 : b0 + nbg, :, 0:64] = _wrap16(s16, nbg)
            idx_all[c, b0 : b0 + nbg, :, 64:128] = _wrap16(d16, nbg)
            idx_all[c, b0 : b0 + nbg, :, 128:192] = _wrap16(c16, nbg)
            b0 += nbg

    # molecule one-hot: oh[c, t, n, m] = 1 iff batch of node (c, t*128+n)
    # equals molw0[c] + m, with molw0 the core's static 1024-mol window.
    molw0 = []
    oh = np.zeros((N_CORES, NT, 128, MOLW), np.float32)
    for c in range(N_CORES):
        bl = batch[c * NPC : (c + 1) * NPC]
        w0 = min((bl[0] // 512) * 512, MOLS - MOLW)
        molw0.append(int(w0))
        assert bl[-1] - w0 < MOLW
        tt = np.arange(NPC) // 128
        nn = np.arange(NPC) % 128
        oh[c, tt, nn, bl - w0] = 1.0
    tables = dict(
        cc_cat=cc,
        wu2=np.ascontiguousarray(Wu[80:144]),
        w1=np.asarray(inputs["W1"], np.float32),
        w2=np.asarray(inputs["W2"], np.float32),
        b1=np.asarray(inputs["b1"], np.float32).reshape(128, 1),
        b2=np.full((128, 1), float(np.asarray(inputs["b2"]).reshape(-1)[0]), np.float32),
    )
    per_core_inputs = []
    for c in range(N_CORES):
        m = dict(tables)
        m["idx_all"] = idx_all[c]
        m["oh"] = oh[c]
        per_core_inputs.append(m)
    return per_core_inputs, nb, nbtot, tuple(molw0)


def _build(nb, nbtot, molw0):
    nc = bacc.Bacc(
        "TRN2", target_bir_lowering=False, debug=False, num_devices=N_CORES,
        num_swdge_queues=4,
    )
    t_cc = nc.dram_tensor("cc_cat", [477, 128], F32, kind="ExternalInput")
    t_wu2 = nc.dram_tensor("wu2", [64, 64], F32, kind="ExternalInput")
    t_w1 = nc.dram_tensor("w1", [64, 128], F32, kind="ExternalInput")
    t_w2 = nc.dram_tensor("w2", [128, 1], F32, kind="ExternalInput")
    t_b1 = nc.dram_tensor("b1", [128, 1], F32, kind="ExternalInput")
    t_b2 = nc.dram_tensor("b2", [128, 1], F32, kind="ExternalInput")
    t_idx = nc.dram_tensor("idx_all", [nbtot, 128, 192], I16, kind="ExternalInput")
    t_oh = nc.dram_tensor("oh", [NT, 128, MOLW], F32, kind="ExternalInput")
    t_out = nc.dram_tensor("out", [2048], F32, kind="ExternalOutput")
    DBG = os.environ.get("DMPNN_DEBUG") == "1"
    if DBG:
        t_dagg = nc.dram_tensor("d_agg", [NPCP, MSG], F32, kind="ExternalOutput")
        t_daggw = nc.dram_tensor("d_aggw", [NPCP, MSG], F32, kind="ExternalOutput")
        t_daggwf = nc.dram_tensor("d_aggwf", [NPCP, MSG], F32, kind="ExternalOutput")
        t_dbase = nc.dram_tensor("d_base", [128, TB * MSG], F32, kind="ExternalOutput")
        t_dmsg = nc.dram_tensor("d_msg", [128, TB * MSG], F32, kind="ExternalOutput")
        t_dgath = nc.dram_tensor("d_gath", [128, TB * MSG], F32, kind="ExternalOutput")

    groups = []
    b0 = 0
    for g in range(N_CHUNKS):
        groups.append((b0, nb[g]))
        b0 += nb[g]

    with tile.TileContext(nc) as tc:
        with (
            tc.tile_pool(name="dram", bufs=1, space="DRAM") as dram,
            tc.tile_pool(name="const", bufs=1) as constp,
            tc.tile_pool(name="sb", bufs=4) as sb,
            tc.tile_pool(name="nsb", bufs=2) as nsb,
            tc.tile_pool(name="cmb", bufs=1) as cmbp,
            tc.tile_pool(name="nodes", bufs=2) as nodes,
            tc.tile_pool(name="psum", bufs=2, space="PSUM") as psum,
            tc.tile_pool(name="psum1", bufs=1, space="PSUM") as psum1,
        ):
            accs = []
            for a in range(NACC):
                acc_t = dram.tile([NPCP, MSG], F32, tag=f"acc{a}", name=f"acc{a}")
                accs.append(acc_t)
            agg = dram.tile([NPCP, MSG], F32)            # combined node slice
            aggw_s = dram.tile([NPCP, MSG], F32)
            aggw_f = dram.tile([FULL_ROWS, MSG], F32)
            base_u = dram.tile([nbtot, 128, TB * MSG], F32)
            molg_in = dram.tile([64, MOLW], F32)
            molg_out = dram.tile([N_CORES * 64, MOLW], F32)

            ident = constp.tile([128, 128], F32)
            make_identity(nc, ident[:])
            wu2 = constp.tile([64, 64], F32)
            nc.sync.dma_start(wu2[:], t_wu2[:, :])
            w1 = constp.tile([64, 128], F32)
            nc.sync.dma_start(w1[:], t_w1[:, :])
            w2 = constp.tile([128, 1], F32)
            nc.sync.dma_start(w2[:], t_w2[:, :])
            b1 = constp.tile([128, 1], F32)
            nc.sync.dma_start(b1[:], t_b1[:, :])
            b2v = constp.tile([128, 1], F32)
            nc.sync.dma_start(b2v[:], t_b2[:, :])
            ZC = NPCP * MSG // 256  # 3136
            zeros = constp.tile([128, ZC], F32)
            nc.vector.memset(zeros[:], 0.0)

            def clear(buf, rows):
                flat = buf[:].rearrange("(p t) f -> p (t f)", p=128)
                cols = rows * MSG // 128
                for o in range(0, cols, ZC):
                    w = min(ZC, cols - o)
                    nc.sync.dma_start(flat[:, o : o + w], zeros[:, :w])

            def edge_pass(rnd, dbg=False):
                for a in range(NACC):
                    clear(accs[a], NPCP)
                bi_ = 0
                for g, (gb0, gnb) in enumerate(groups):
                    for b in range(gb0, gb0 + gnb):
                        acc = accs[bi_ % NACC]
                        bi_ += 1
                        msg = sb.tile([128, TB, MSG], F32, tag="msg")
                        if rnd == 0:
                            idxt = sb.tile([128, 128], I16, tag="idxt")
                            nc.sync.dma_start(idxt[:], t_idx[b][:, 64:192])
                            gath = sb.tile([128, TB, 128], F32, tag="gath")
                            nc.gpsimd.dma_gather(
                                gath[:, :, :], t_cc[:, :], idxt[:, 64:128],
                                BLOCK, BLOCK, 128, queue_num=bi_ % 2,
                            )
                            nc.scalar.activation(
                                msg[:, :, :], gath[:, :, 0:64],
                                mybir.ActivationFunctionType.Relu,
                            )
                            nc.scalar.dma_start(
                                base_u[b].rearrange("p (a b) -> p a b", b=MSG),
                                gath[:, :, 64:128],
                            )
                            nc.gpsimd.dma_scatter_add(
                                acc[:], msg[:, :, :], idxt[:, 0:64],
                                BLOCK, BLOCK, MSG, queue_num=2 + bi_ % 2,
                            )
                        else:
                            idxt = sb.tile([128, 128], I16, tag="idxt")
                            nc.sync.dma_start(idxt[:], t_idx[b][:, 0:128])
                            gath = sb.tile([128, TB, MSG], F32, tag="gath")
                            nc.gpsimd.dma_gather(
                                gath[:, :, :],
                                aggw_f[g * CHUNK_ROWS : (g + 1) * CHUNK_ROWS, :],
                                idxt[:, 0:64], BLOCK, BLOCK, MSG,
                                queue_num=bi_ % 2,
                            )
                            bb = sb.tile([128, TB, MSG], F32, tag="base")
                            nc.scalar.dma_start(
                                bb[:].rearrange("p a b -> p (a b)"), base_u[b]
                            )
                            summ = sb.tile([128, TB * MSG], F32, tag="summ")
                            nc.vector.tensor_tensor(
                                out=summ[:],
                                in0=gath[:].rearrange("p a b -> p (a b)"),
                                in1=bb[:].rearrange("p a b -> p (a b)"),
                                op=mybir.AluOpType.add,
                            )
                            nc.scalar.activation(
                                msg[:].rearrange("p a b -> p (a b)"), summ[:],
                                mybir.ActivationFunctionType.Relu,
                            )
                            scq = 2 + bi_ % 2
                            if dbg and b == 0:
                                nc.sync.dma_start(t_dbase[:, :], bb[:].rearrange("p a b -> p (a b)"))
                                nc.sync.dma_start(t_dmsg[:, :], msg[:].rearrange("p a b -> p (a b)"))
                                nc.sync.dma_start(t_dgath[:, :], gath[:].rearrange("p a b -> p (a b)"))
                            nc.gpsimd.dma_scatter_add(
                                acc[:], msg[:, :, :], idxt[:, 64:128],
                                BLOCK, BLOCK, MSG, queue_num=scq,
                            )

            def combine():
                # agg = sum(accs), tree reduce per chunk
                H = ZC // 2  # 1568
                for o in range(0, 4 * H, H):
                    parts = []
                    for a in range(NACC):
                        pt = cmbp.tile([128, H], F32, tag=f"cmb{a}", name=f"cmb{a}")
                        nc.sync.dma_start(
                            pt[:],
                            accs[a][:].rearrange("(p t) f -> p (t f)", p=128)[
                                :, o : o + H
                            ],
                        )
                        parts.append(pt)
                    lvl = 0
                    while len(parts) > 1:
                        nxt = []
                        for j in range(0, len(parts), 2):
                            sx = cmbp.tile(
                                [128, H], F32, tag=f"cmbs{lvl}_{j}",
                                name=f"cmbs{lvl}_{j}",
                            )
                            nc.vector.tensor_tensor(
                                out=sx[:], in0=parts[j][:], in1=parts[j + 1][:],
                                op=mybir.AluOpType.add,
                            )
                            nxt.append(sx)
                        parts = nxt
                        lvl += 1
                    nc.sync.dma_start(
                        agg[:].rearrange("(p t) f -> p (t f)", p=128)[:, o : o + H],
                        parts[0][:],
                    )

            def node_stage():
                spans = [(q * 512, 512) for q in range(NPCP // 512)]
                if NPCP % 512:
                    spans.append((NPCP - NPCP % 512, NPCP % 512))
                for r0, rn in spans:
                    kt = rn // 128
                    a4 = nodes.tile([128, 4, MSG], F32, tag="a4")
                    nc.sync.dma_start(
                        a4[:, :kt, :],
                        agg[r0 : r0 + rn, :].rearrange("(t p) f -> p t f", p=128),
                    )
                    atp = psum.tile([64, 512], F32, tag="atp", space="PSUM")
                    for k in range(kt):
                        nc.tensor.transpose(
                            atp[:, k * 128 : (k + 1) * 128], a4[:, k, :], ident[:]
                        )
                    ats = nodes.tile([64, 512], F32, tag="ats")
                    nc.vector.tensor_copy(ats[:, : kt * 128], atp[:, : kt * 128])
                    w4 = nodes.tile([128, 4, MSG], F32, tag="w4")
                    for k in range(kt):
                        wp = psum.tile([128, MSG], F32, tag="wp", space="PSUM")
                        nc.tensor.matmul(
                            wp[:], ats[:, k * 128 : (k + 1) * 128], wu2[:],
                            start=True, stop=True,
                        )
                        nc.vector.tensor_copy(w4[:, k, :], wp[:])
                    nc.sync.dma_start(
                        aggw_s[r0 : r0 + rn, :].rearrange("(t p) f -> p t f", p=128),
                        w4[:, :kt, :],
                    )

            def dump(dst_dram, src_dram, rows):
                for t0 in range(0, rows - 511, 512):
                    dt_ = nsb.tile([128, 4, MSG], F32, tag="dump")
                    nc.sync.dma_start(
                        dt_[:, :, :],
                        src_dram[t0 : t0 + 512, :].rearrange("(t p) f -> p t f", p=128),
                    )
                    nc.sync.dma_start(
                        dst_dram[t0 : t0 + 512, :].rearrange("(t p) f -> p t f", p=128),
                        dt_[:, :, :],
                    )

            edge_pass(0)
            for _r in range(ROUNDS):
                combine()
                if DBG and _r == 0:
                    dump(t_dagg, agg, NPCP)
                node_stage()
                if DBG and _r == 0:
                    dump(t_daggw, aggw_s, NPCP)
                nc.gpsimd.collective_compute(
                    "AllGather", mybir.AluOpType.bypass,
                    replica_groups=[list(range(N_CORES))],
                    ins=[aggw_s[:]], outs=[aggw_f[:]],
                )
                if DBG and _r == 0:
                    dump(t_daggwf, aggw_f, NPCP)
                edge_pass(1, dbg=DBG and _r == 0)
            combine()

            # molecules: molT_win[f, m] = sum_t ns_t^T @ oh_t  (PSUM-resident)
            molp = psum1.tile([64, MOLW], F32, tag="molp", space="PSUM")
            for t in range(NT):
                nst = nsb.tile([128, MSG], F32, tag="nst")
                nc.sync.dma_start(nst[:], agg[t * 128 : (t + 1) * 128, :])
                oht = nsb.tile([128, MOLW], F32, tag="oht")
                nc.sync.dma_start(oht[:], t_oh[t])
                for h in range(MOLW // 512):
                    nc.tensor.matmul(
                        molp[:, h * 512 : (h + 1) * 512], nst[:],
                        oht[:, h * 512 : (h + 1) * 512],
                        start=(t == 0), stop=(t == NT - 1),
                    )
            molw_sb = nodes.tile([64, MOLW], F32, tag="molw")
            nc.vector.tensor_copy(molw_sb[:], molp[:])
            nc.sync.dma_start(molg_in[:], molw_sb[:])
            nc.gpsimd.collective_compute(
                "AllGather", mybir.AluOpType.bypass,
                replica_groups=[list(range(N_CORES))],
                ins=[molg_in[:]], outs=[molg_out[:]],
            )
            molT = nodes.tile([64, MOLS], F32, tag="molT")
            nc.vector.memset(molT[:], 0.0)
            for c in range(N_CORES):
                gc = nodes.tile([64, MOLW], F32, tag="molw")
                nc.sync.dma_start(gc[:], molg_out[c * 64 : (c + 1) * 64, :])
                w0 = molw0[c]
                nc.vector.tensor_tensor(
                    out=molT[:, w0 : w0 + MOLW],
                    in0=molT[:, w0 : w0 + MOLW],
                    in1=gc[:],
                    op=mybir.AluOpType.add,
                )

            # readout: hT = relu(W1^T @ molT + b1); out = hT^T @ W2 + b2
            hT = nodes.tile([128, MOLS], F32, tag="hT")
            for q in range(MOLS // 512):
                hp = psum.tile([128, 512], F32, tag="atp", space="PSUM")
                nc.tensor.matmul(
                    hp[:], w1[:], molT[:, q * 512 : (q + 1) * 512],
                    start=True, stop=True,
                )
                nc.scalar.activation(
                    hT[:, q * 512 : (q + 1) * 512], hp[:],
                    mybir.ActivationFunctionType.Relu, bias=b1[:, :1],
                )
            ot = nodes.tile([128, 16], F32, tag="ot")
            for q in range(16):
                op_ = psum.tile([128, 1], F32, tag="wp", space="PSUM")
                nc.tensor.matmul(
                    op_[:], hT[:, q * 128 : (q + 1) * 128], w2[:],
                    start=True, stop=True,
                )
                nc.vector.tensor_copy(ot[:, q : q + 1], op_[:])
            ob = nodes.tile([128, 16], F32, tag="ob")
            nc.vector.tensor_scalar_add(ob[:], ot[:], b2v[:, :1])
            nc.sync.dma_start(t_out[:].rearrange("(t p) -> p t", p=128), ob[:])

    nc.compile()
    return nc


def kernel(**inputs):
    per_core_inputs, nb, nbtot, molw0 = _prep(inputs)
    key = (tuple(nb), molw0)
    if key not in _CACHE:
        _CACHE[key] = _build(nb, nbtot, molw0)
    nc = _CACHE[key]
    res = bass_utils.run_bass_kernel_spmd(
        nc, per_core_inputs, core_ids=list(range(N_CORES))
    )
    return np.asarray(res.results[0]["out"], np.float32)


# revision 26
# speedup vs baseline: 2.3274x; 2.3274x over previous
"""Trainium2 Bass kernel for BasicDMPNN (gnn_message_passing).

Strategy:
  - Nodes are partitioned contiguously across 8 cores (12500 each); every
    edge is owned by the core that owns its dst node, so the segment-sum
    (dma_scatter_add) is core-local.
  - The edge MLPs fold into tiny tables:
      msg_0[e]   = relu(Ci[code[e]])
      msg_r[e]   = relu(Cu[code[e]] + (agg_{r-1} @ Wu2)[src[e]])
    with code[e] = 4*x[src[e]] + edge_attr[e] (476 entries), because
    ab = [atom_table[x[src]], bond_table[ea]] enters the MLPs linearly.
  - Per round: each core computes aggW = agg @ Wu2 for its node slice
    (PE), AllGather makes the full table visible to all cores, then the
    edge pass is dma_gather(aggW)[src] + base + relu + dma_scatter_add
    by dst.
  - dma_gather / dma_scatter_add are limited to ~1024 indices per call
    (SWDGE ring) and scatter-add loses updates for duplicate indices
    within one call; calls targeting the same tensor serialize (WAW) and
    are then exact. So blocks are 1024 edges, each dst appears at most
    once per block (host round-robin), and blocks rotate over 4
    accumulator tensors (summed at the end of the round) to keep four
    independent WAW chains.
  - base[e] = Cu[code[e]] is materialized once (round 0, bf16) and
    streamed sequentially in rounds 1-4.
  - Molecule readout: one-hot matmul (host-built selection matrices) into
    a per-core 1024-mol window held in PSUM, AllGather of the transposed
    partials, combine at static per-core offsets, then the MLP head.
"""

import os

import numpy as np

import concourse.bacc as bacc
import concourse.bass as bass
import concourse.mybir as mybir
import concourse.tile as tile
from concourse import bass_utils
from concourse.masks import make_identity

N_CORES = 8
N_NODES = 100000
NPC = 12500          # nodes per core
NPCP = 12544         # padded node slice (98 * 128); rows >= 12500 are scratch
N_CHUNKS = 4         # src chunks so gather indices fit int16
CHUNK_ROWS = 2 * NPCP  # 25088 rows per chunk of the allgathered table
FULL_ROWS = N_CORES * NPCP  # 100352
MOLS = 2048
MOLW = 1024          # per-core mol window (each core's mols span < 512)
MSG = 64
BLOCK = 1024
TB = BLOCK // 128    # 8
NACC = 4             # parallel scatter accumulator tensors
NT = NPCP // 128     # 98 node tiles per core
ROUNDS = int(os.environ.get("DMPNN_ROUNDS", "4"))
F32 = mybir.dt.float32
BF16 = mybir.dt.bfloat16
I16 = mybir.dt.int16

_CACHE = {}


def _wrap16(idx, nblocks):
    """[nblocks*B] int -> [nblocks, 128, B//16] int16: index i of a block
    sits at [i % 16, i // 16], replicated across the 8 Q7 core groups."""
    b = idx.size // nblocks
    v = np.transpose(idx.reshape(nblocks, b // 16, 16), (0, 2, 1)).astype(np.int16)
    return np.tile(v, (1, 8, 1))


def _prep(inputs):
    x = np.asarray(inputs["x"]).astype(np.int64)
    ea = np.asarray(inputs["edge_attr"]).astype(np.int64)
    ei = np.asarray(inputs["edge_index"]).astype(np.int64)
    batch = np.asarray(inputs["batch"]).astype(np.int64)
    atom_table = np.asarray(inputs["atom_table"], np.float32)
    bond_table = np.asarray(inputs["bond_table"], np.float32)
    Wi = np.asarray(inputs["Wi"], np.float32)
    bi = np.asarray(inputs["bi"], np.float32)
    Wu = np.asarray(inputs["Wu"], np.float32)
    bu = np.asarray(inputs["bu"], np.float32)

    src, dst = ei[0], ei[1]
    a_i = atom_table @ Wi[:64]
    b_i = bond_table @ Wi[64:80]
    a_u = atom_table @ Wu[:64]
    b_u = bond_table @ Wu[64:80]
    Ci = (a_i[:, None, :] + b_i[None, :, :] + bi).reshape(476, 64)
    Cu = (a_u[:, None, :] + b_u[None, :, :] + bu).reshape(476, 64)
    cc = np.zeros((477, 128), np.float32)
    cc[:476, :64] = Ci
    cc[:476, 64:] = Cu

    code = 4 * x[src] + ea
    owner = dst // NPC
    chunk = src // (2 * NPC)
    grow = (src // NPC) * NPCP + (src % NPC)

    per_core = [[None] * N_CHUNKS for _ in range(N_CORES)]
    cnt = np.zeros((N_CORES, N_CHUNKS), np.int64)
    maxdeg = np.zeros((N_CORES, N_CHUNKS), np.int64)
    for c in range(N_CORES):
        m = owner == c
        ch = chunk[m]
        for g in range(N_CHUNKS):
            sel = np.nonzero(m)[0][ch == g]
            per_core[c][g] = sel
            cnt[c, g] = len(sel)
            if len(sel):
                maxdeg[c, g] = np.bincount(dst[sel] - c * NPC).max()
    nb = [
        int(max(np.ceil(cnt[:, g].max() * 1.03 / BLOCK), maxdeg[:, g].max()))
        for g in range(N_CHUNKS)
    ]
    nbtot = sum(nb)

    # packed per-block indices: cols 0:64 src, 64:128 dst, 128:192 code
    idx_all = np.zeros((N_CORES, nbtot, 128, 192), np.int16)
    for c in range(N_CORES):
        b0 = 0
        for g in range(N_CHUNKS):
            sel = per_core[c][g]
            nbg = nb[g]
            n = nbg * BLOCK
            s16 = np.full(n, 0, np.int64)
            d16 = np.full(n, NPCP - 1, np.int64)   # pad -> scratch row
            c16 = np.full(n, 476, np.int64)        # pad -> zero table row
            if len(sel):
                dl = dst[sel] - c * NPC
                order = np.argsort(dl, kind="stable")
                sel = sel[order]
                dl = dl[order]
                deg = np.bincount(dl, minlength=NPC)
                first = np.zeros(len(dl), bool)
                first[0] = True
                first[1:] = dl[1:] != dl[:-1]
                rank = np.arange(len(dl)) - np.repeat(
                    np.nonzero(first)[0], deg[deg > 0]
                )
                start = np.cumsum(deg) - deg
                blk = (start[dl] + rank) % nbg
                bo = np.argsort(blk, kind="stable")
                fills = np.bincount(blk, minlength=nbg)
                assert fills.max() <= BLOCK, (c, g, fills.max())
                pos = np.concatenate(
                    [b * BLOCK + np.arange(f) for b, f in enumerate(fills)]
                )
                s16[pos] = grow[sel[bo]] - g * CHUNK_ROWS
                d16[pos] = dl[bo]
                c16[pos] = code[sel[bo]]
            idx_all[c, b0 : b0 + nbg, :, 0:64] = _wrap16(s16, nbg)
            idx_all[c, b0 : b0 + nbg, :, 64:128] = _wrap16(d16, nbg)
            idx_all[c, b0 : b0 + nbg, :, 128:192] = _wrap16(c16, nbg)
            b0 += nbg

    # molecule one-hot: oh[c, t, n, m] = 1 iff batch of node (c, t*128+n)
    # equals molw0[c] + m, with molw0 the core's static 1024-mol window.
    molw0 = []
    oh = np.zeros((N_CORES, NT, 128, MOLW), np.float32)
    for c in range(N_CORES):
        bl = batch[c * NPC : (c + 1) * NPC]
        w0 = min((bl[0] // 512) * 512, MOLS - MOLW)
        molw0.append(int(w0))
        assert bl[-1] - w0 < MOLW
        tt = np.arange(NPC) // 128
        nn = np.arange(NPC) % 128
        oh[c, tt, nn, bl - w0] = 1.0
    tables = dict(
        cc_cat=cc,
        wu2=np.ascontiguousarray(Wu[80:144]),
        w1=np.asarray(inputs["W1"], np.float32),
        w2=np.asarray(inputs["W2"], np.float32),
        b1=np.asarray(inputs["b1"], np.float32).reshape(128, 1),
        b2=np.full((128, 1), float(np.asarray(inputs["b2"]).reshape(-1)[0]), np.float32),
    )
    per_core_inputs = []
    for c in range(N_CORES):
        m = dict(tables)
        m["idx_all"] = idx_all[c]
        m["oh"] = oh[c]
        per_core_inputs.append(m)
    return per_core_inputs, nb, nbtot, tuple(molw0)


def _build(nb, nbtot, molw0):
    nc = bacc.Bacc(
        "TRN2", target_bir_lowering=False, debug=False, num_devices=N_CORES,
        num_swdge_queues=4,
    )
    t_cc = nc.dram_tensor("cc_cat", [477, 128], F32, kind="ExternalInput")
    t_wu2 = nc.dram_tensor("wu2", [64, 64], F32, kind="ExternalInput")
    t_w1 = nc.dram_tensor("w1", [64, 128], F32, kind="ExternalInput")
    t_w2 = nc.dram_tensor("w2", [128, 1], F32, kind="ExternalInput")
    t_b1 = nc.dram_tensor("b1", [128, 1], F32, kind="ExternalInput")
    t_b2 = nc.dram_tensor("b2", [128, 1], F32, kind="ExternalInput")
    t_idx = nc.dram_tensor("idx_all", [nbtot, 128, 192], I16, kind="ExternalInput")
    t_oh = nc.dram_tensor("oh", [NT, 128, MOLW], F32, kind="ExternalInput")
    t_out = nc.dram_tensor("out", [2048], F32, kind="ExternalOutput")
    DBG = os.environ.get("DMPNN_DEBUG") == "1"
    if DBG:
        t_dagg = nc.dram_tensor("d_agg", [NPCP, MSG], F32, kind="ExternalOutput")
        t_daggw = nc.dram_tensor("d_aggw", [NPCP, MSG], F32, kind="ExternalOutput")
        t_daggwf = nc.dram_tensor("d_aggwf", [NPCP, MSG], F32, kind="ExternalOutput")
        t_dbase = nc.dram_tensor("d_base", [128, TB * MSG], F32, kind="ExternalOutput")
        t_dmsg = nc.dram_tensor("d_msg", [128, TB * MSG], F32, kind="ExternalOutput")
        t_dgath = nc.dram_tensor("d_gath", [128, TB * MSG], F32, kind="ExternalOutput")

    groups = []
    b0 = 0
    for g in range(N_CHUNKS):
        groups.append((b0, nb[g]))
        b0 += nb[g]

    with tile.TileContext(nc) as tc:
        with (
            tc.tile_pool(name="dram", bufs=1, space="DRAM") as dram,
            tc.tile_pool(name="const", bufs=1) as constp,
            tc.tile_pool(name="sb", bufs=4) as sb,
            tc.tile_pool(name="nsb", bufs=2) as nsb,
            tc.tile_pool(name="cmb", bufs=1) as cmbp,
            tc.tile_pool(name="nodes", bufs=2) as nodes,
            tc.tile_pool(name="psum", bufs=2, space="PSUM") as psum,
            tc.tile_pool(name="psum1", bufs=1, space="PSUM") as psum1,
        ):
            accs = []
            for a in range(NACC):
                acc_t = dram.tile([NPCP, MSG], F32, tag=f"acc{a}", name=f"acc{a}")
                accs.append(acc_t)
            agg = dram.tile([NPCP, MSG], F32)            # combined node slice
            aggw_s = dram.tile([NPCP, MSG], F32)
            aggw_f = dram.tile([FULL_ROWS, MSG], F32)
            base_u = dram.tile([nbtot, 128, TB * MSG], F32)
            molg_in = dram.tile([64, MOLW], F32)
            molg_out = dram.tile([N_CORES * 64, MOLW], F32)

            ident = constp.tile([128, 128], F32)
            make_identity(nc, ident[:])
            wu2 = constp.tile([64, 64], F32)
            nc.sync.dma_start(wu2[:], t_wu2[:, :])
            w1 = constp.tile([64, 128], F32)
            nc.sync.dma_start(w1[:], t_w1[:, :])
            w2 = constp.tile([128, 1], F32)
            nc.sync.dma_start(w2[:], t_w2[:, :])
            b1 = constp.tile([128, 1], F32)
            nc.sync.dma_start(b1[:], t_b1[:, :])
            b2v = constp.tile([128, 1], F32)
            nc.sync.dma_start(b2v[:], t_b2[:, :])
            ZC = NPCP * MSG // 256  # 3136
            zeros = constp.tile([128, ZC], F32)
            nc.vector.memset(zeros[:], 0.0)

            def clear(buf, rows):
                flat = buf[:].rearrange("(p t) f -> p (t f)", p=128)
                cols = rows * MSG // 128
                for o in range(0, cols, ZC):
                    w = min(ZC, cols - o)
                    nc.sync.dma_start(flat[:, o : o + w], zeros[:, :w])

            def edge_pass(rnd, dbg=False):
                for a in range(NACC):
                    clear(accs[a], NPCP)
                bi_ = 0
                for g, (gb0, gnb) in enumerate(groups):
                    for b in range(gb0, gb0 + gnb):
                        acc = accs[bi_ % NACC]
                        bi_ += 1
                        msg = sb.tile([128, TB, MSG], F32, tag="msg")
                        if rnd == 0:
                            idxt = sb.tile([128, 128], I16, tag="idxt")
                            nc.sync.dma_start(idxt[:], t_idx[b][:, 64:192])
                            gath = sb.tile([128, TB, 128], F32, tag="gath")
                            nc.gpsimd.dma_gather(
                                gath[:, :, :], t_cc[:, :], idxt[:, 64:128],
                                BLOCK, BLOCK, 128, queue_num=bi_ % 2,
                            )
                            nc.scalar.activation(
                                msg[:, :, :], gath[:, :, 0:64],
                                mybir.ActivationFunctionType.Relu,
                            )
                            nc.scalar.dma_start(
                                base_u[b].rearrange("p (a b) -> p a b", b=MSG),
                                gath[:, :, 64:128],
                            )
                            nc.gpsimd.dma_scatter_add(
                                acc[:], msg[:, :, :], idxt[:, 0:64],
                                BLOCK, BLOCK, MSG, queue_num=2 + bi_ % 2,
                            )
                        else:
                            idxt = sb.tile([128, 128], I16, tag="idxt")
                            nc.sync.dma_start(idxt[:], t_idx[b][:, 0:128])
                            gath = sb.tile([128, TB, MSG], F32, tag="gath")
                            nc.gpsimd.dma_gather(
                                gath[:, :, :],
                                aggw_f[g * CHUNK_ROWS : (g + 1) * CHUNK_ROWS, :],
                                idxt[:, 0:64], BLOCK, BLOCK, MSG,
                                queue_num=bi_ % 2,
                            )
                            bb = sb.tile([128, TB, MSG], F32, tag="base")
                            nc.scalar.dma_start(
                                bb[:].rearrange("p a b -> p (a b)"), base_u[b]
                            )
                            summ = sb.tile([128, TB * MSG], F32, tag="summ")
                            nc.vector.tensor_tensor(
                                out=summ[:],
                                in0=gath[:].rearrange("p a b -> p (a b)"),
                                in1=bb[:].rearrange("p a b -> p (a b)"),
                                op=mybir.AluOpType.add,
                            )
                            nc.scalar.activation(
                                msg[:].rearrange("p a b -> p (a b)"), summ[:],
                                mybir.ActivationFunctionType.Relu,
                            )
                            scq = 2 + bi_ % 2
                            if dbg and b == 0:
                                nc.sync.dma_start(t_dbase[:, :], bb[:].rearrange("p a b -> p (a b)"))
                                nc.sync.dma_start(t_dmsg[:, :], msg[:].rearrange("p a b -> p (a b)"))
                                nc.sync.dma_start(t_dgath[:, :], gath[:].rearrange("p a b -> p (a b)"))
                            nc.gpsimd.dma_scatter_add(
                                acc[:], msg[:, :, :], idxt[:, 64:128],
                                BLOCK, BLOCK, MSG, queue_num=scq,
                            )

            def combine():
                # agg = sum(accs), tree reduce per chunk
                H = ZC // 2  # 1568
                for o in range(0, 4 * H, H):
                    parts = []
                    for a in range(NACC):
                        pt = cmbp.tile([128, H], F32, tag=f"cmb{a}", name=f"cmb{a}")
                        nc.sync.dma_start(
                            pt[:],
                            accs[a][:].rearrange("(p t) f -> p (t f)", p=128)[
                                :, o : o + H
                            ],
                        )
                        parts.append(pt)
                    lvl = 0
                    while len(parts) > 1:
                        nxt = []
                        for j in range(0, len(parts), 2):
                            sx = cmbp.tile(
                                [128, H], F32, tag=f"cmbs{lvl}_{j}",
                                name=f"cmbs{lvl}_{j}",
                            )
                            nc.vector.tensor_tensor(
                                out=sx[:], in0=parts[j][:], in1=parts[j + 1][:],
                                op=mybir.AluOpType.add,
                            )
                            nxt.append(sx)
                        parts = nxt
                        lvl += 1
                    nc.sync.dma_start(
                        agg[:].rearrange("(p t) f -> p (t f)", p=128)[:, o : o + H],
                        parts[0][:],
                    )

            def node_stage():
                spans = [(q * 512, 512) for q in range(NPCP // 512)]
                if NPCP % 512:
                    spans.append((NPCP - NPCP % 512, NPCP % 512))
                for r0, rn in spans:
                    kt = rn // 128
                    a4 = nodes.tile([128, 4, MSG], F32, tag="a4")
                    nc.sync.dma_start(
                        a4[:, :kt, :],
                        agg[r0 : r0 + rn, :].rearrange("(t p) f -> p t f", p=128),
                    )
                    atp = psum.tile([64, 512], F32, tag="atp", space="PSUM")
                    for k in range(kt):
                        nc.tensor.transpose(
                            atp[:, k * 128 : (k + 1) * 128], a4[:, k, :], ident[:]
                        )
                    ats = nodes.tile([64, 512], F32, tag="ats")
                    nc.vector.tensor_copy(ats[:, : kt * 128], atp[:, : kt * 128])
                    w4 = nodes.tile([128, 4, MSG], F32, tag="w4")
                    for k in range(kt):
                        wp = psum.tile([128, MSG], F32, tag="wp", space="PSUM")
                        nc.tensor.matmul(
                            wp[:], ats[:, k * 128 : (k + 1) * 128], wu2[:],
                            start=True, stop=True,
                        )
                        nc.vector.tensor_copy(w4[:, k, :], wp[:])
                    nc.sync.dma_start(
                        aggw_s[r0 : r0 + rn, :].rearrange("(t p) f -> p t f", p=128),
                        w4[:, :kt, :],
                    )

            def dump(dst_dram, src_dram, rows):
                for t0 in range(0, rows - 511, 512):
                    dt_ = nsb.tile([128, 4, MSG], F32, tag="dump")
                    nc.sync.dma_start(
                        dt_[:, :, :],
                        src_dram[t0 : t0 + 512, :].rearrange("(t p) f -> p t f", p=128),
                    )
                    nc.sync.dma_start(
                        dst_dram[t0 : t0 + 512, :].rearrange("(t p) f -> p t f", p=128),
                        dt_[:, :, :],
                    )

            edge_pass(0)
            for _r in range(ROUNDS):
                combine()
                if DBG and _r == 0:
                    dump(t_dagg, agg, NPCP)
                node_stage()
                if DBG and _r == 0:
                    dump(t_daggw, aggw_s, NPCP)
                nc.gpsimd.collective_compute(
                    "AllGather", mybir.AluOpType.bypass,
                    replica_groups=[list(range(N_CORES))],
                    ins=[aggw_s[:]], outs=[aggw_f[:]],
                )
                if DBG and _r == 0:
                    dump(t_daggwf, aggw_f, NPCP)
                edge_pass(1, dbg=DBG and _r == 0)
            combine()

            # molecules: molT_win[f, m] = sum_t ns_t^T @ oh_t  (PSUM-resident)
            molp = psum1.tile([64, MOLW], F32, tag="molp", space="PSUM")
            for t in range(NT):
                nst = nsb.tile([128, MSG], F32, tag="nst")
                nc.sync.dma_start(nst[:], agg[t * 128 : (t + 1) * 128, :])
                oht = nsb.tile([128, MOLW], F32, tag="oht")
                nc.sync.dma_start(oht[:], t_oh[t])
                for h in range(MOLW // 512):
                    nc.tensor.matmul(
                        molp[:, h * 512 : (h + 1) * 512], nst[:],
                        oht[:, h * 512 : (h + 1) * 512],
                        start=(t == 0), stop=(t == NT - 1),
                    )
            molw_sb = nodes.tile([64, MOLW], F32, tag="molw")
            nc.vector.tensor_copy(molw_sb[:], molp[:])
            nc.sync.dma_start(molg_in[:], molw_sb[:])
            nc.gpsimd.collective_compute(
                "AllGather", mybir.AluOpType.bypass,
                replica_groups=[list(range(N_CORES))],
                ins=[molg_in[:]], outs=[molg_out[:]],
            )
            molT = nodes.tile([64, MOLS], F32, tag="molT")
            nc.vector.memset(molT[:], 0.0)
            for c in range(N_CORES):
                gc = nodes.tile([64, MOLW], F32, tag="molw")
                nc.sync.dma_start(gc[:], molg_out[c * 64 : (c + 1) * 64, :])
                w0 = molw0[c]
                nc.vector.tensor_tensor(
                    out=molT[:, w0 : w0 + MOLW],
                    in0=molT[:, w0 : w0 + MOLW],
                    in1=gc[:],
                    op=mybir.AluOpType.add,
                )

            # readout: hT = relu(W1^T @ molT + b1); out = hT^T @ W2 + b2
            hT = nodes.tile([128, MOLS], F32, tag="hT")
            for q in range(MOLS // 512):
                hp = psum.tile([128, 512], F32, tag="atp", space="PSUM")
                nc.tensor.matmul(
                    hp[:], w1[:], molT[:, q * 512 : (q + 1) * 512],
                    start=True, stop=True,
                )
                nc.scalar.activation(
                    hT[:, q * 512 : (q + 1) * 512], hp[:],
                    mybir.ActivationFunctionType.Relu, bias=b1[:, :1],
                )
            ot = nodes.tile([128, 16], F32, tag="ot")
            for q in range(16):
                op_ = psum.tile([128, 1], F32, tag="wp", space="PSUM")
                nc.tensor.matmul(
                    op_[:], hT[:, q * 128 : (q + 1) * 128], w2[:],
                    start=True, stop=True,
                )
                nc.vector.tensor_copy(ot[:, q : q + 1], op_[:])
            ob = nodes.tile([128, 16], F32, tag="ob")
            nc.vector.tensor_scalar_add(ob[:], ot[:], b2v[:, :1])
            nc.sync.dma_start(t_out[:].rearrange("(t p) -> p t", p=128), ob[:])

    nc.compile()
    return nc


def kernel(**inputs):
    per_core_inputs, nb, nbtot, molw0 = _prep(inputs)
    key = (tuple(nb), molw0)
    if key not in _CACHE:
        _CACHE[key] = _build(nb, nbtot, molw0)
    nc = _CACHE[key]
    res = bass_utils.run_bass_kernel_spmd(
        nc, per_core_inputs, core_ids=list(range(N_CORES))
    )
    return np.asarray(res.results[0]["out"], np.float32)


# revision 29
# speedup vs baseline: 2.4074x; 1.0344x over previous
"""Trainium2 Bass kernel for BasicDMPNN (gnn_message_passing).

Strategy:
  - Nodes are partitioned contiguously across 8 cores (12500 each); every
    edge is owned by the core that owns its dst node, so the segment-sum
    (dma_scatter_add) is core-local.
  - The edge MLPs fold into tiny tables:
      msg_0[e]   = relu(Ci[code[e]])
      msg_r[e]   = relu(Cu[code[e]] + (agg_{r-1} @ Wu2)[src[e]])
    with code[e] = 4*x[src[e]] + edge_attr[e] (476 entries), because
    ab = [atom_table[x[src]], bond_table[ea]] enters the MLPs linearly.
  - Per round: each core computes aggW = agg @ Wu2 for its node slice
    (PE), AllGather makes the full table visible to all cores, then the
    edge pass is dma_gather(aggW)[src] + base + relu + dma_scatter_add
    by dst.
  - dma_gather / dma_scatter_add are limited to ~1024 indices per call
    (SWDGE ring) and scatter-add loses updates for duplicate indices
    within one call; calls targeting the same tensor serialize (WAW) and
    are then exact. So blocks are 1024 edges, each dst appears at most
    once per block (host round-robin), and blocks rotate over 4
    accumulator tensors (summed at the end of the round) to keep four
    independent WAW chains.
  - base[e] = Cu[code[e]] is materialized once (round 0, bf16) and
    streamed sequentially in rounds 1-4.
  - Molecule readout: one-hot matmul (host-built selection matrices) into
    a per-core 1024-mol window held in PSUM, AllGather of the transposed
    partials, combine at static per-core offsets, then the MLP head.
"""

import os

import numpy as np

import concourse.bacc as bacc
import concourse.bass as bass
import concourse.mybir as mybir
import concourse.tile as tile
from concourse import bass_utils
from concourse.masks import make_identity

N_CORES = 8
N_NODES = 100000
NPC = 12500          # nodes per core
NPCP = 12544         # padded node slice (98 * 128); rows >= 12500 are scratch
N_CHUNKS = 4         # src chunks so gather indices fit int16
CHUNK_ROWS = 2 * NPCP  # 25088 rows per chunk of the allgathered table
FULL_ROWS = N_CORES * NPCP  # 100352
MOLS = 2048
MOLW = 1024          # per-core mol window (each core's mols span < 512)
MSG = 64
BLOCK = 1024
TB = BLOCK // 128    # 8
NACC = 4             # parallel scatter accumulator tensors
NT = NPCP // 128     # 98 node tiles per core
ROUNDS = int(os.environ.get("DMPNN_ROUNDS", "4"))
QMODE = int(os.environ.get("DMPNN_QMODE", "1"))
F32 = mybir.dt.float32
BF16 = mybir.dt.bfloat16
I16 = mybir.dt.int16

_CACHE = {}


def _wrap16(idx, nblocks):
    """[nblocks*B] int -> [nblocks, 128, B//16] int16: index i of a block
    sits at [i % 16, i // 16], replicated across the 8 Q7 core groups."""
    b = idx.size // nblocks
    v = np.transpose(idx.reshape(nblocks, b // 16, 16), (0, 2, 1)).astype(np.int16)
    return np.tile(v, (1, 8, 1))


def _prep(inputs):
    x = np.asarray(inputs["x"]).astype(np.int64)
    ea = np.asarray(inputs["edge_attr"]).astype(np.int64)
    ei = np.asarray(inputs["edge_index"]).astype(np.int64)
    batch = np.asarray(inputs["batch"]).astype(np.int64)
    atom_table = np.asarray(inputs["atom_table"], np.float32)
    bond_table = np.asarray(inputs["bond_table"], np.float32)
    Wi = np.asarray(inputs["Wi"], np.float32)
    bi = np.asarray(inputs["bi"], np.float32)
    Wu = np.asarray(inputs["Wu"], np.float32)
    bu = np.asarray(inputs["bu"], np.float32)

    src, dst = ei[0], ei[1]
    a_i = atom_table @ Wi[:64]
    b_i = bond_table @ Wi[64:80]
    a_u = atom_table @ Wu[:64]
    b_u = bond_table @ Wu[64:80]
    Ci = (a_i[:, None, :] + b_i[None, :, :] + bi).reshape(476, 64)
    Cu = (a_u[:, None, :] + b_u[None, :, :] + bu).reshape(476, 64)
    cc = np.zeros((477, 128), np.float32)
    cc[:476, :64] = Ci
    cc[:476, 64:] = Cu

    code = 4 * x[src] + ea
    owner = dst // NPC
    chunk = src // (2 * NPC)
    grow = (src // NPC) * NPCP + (src % NPC)

    per_core = [[None] * N_CHUNKS for _ in range(N_CORES)]
    cnt = np.zeros((N_CORES, N_CHUNKS), np.int64)
    maxdeg = np.zeros((N_CORES, N_CHUNKS), np.int64)
    for c in range(N_CORES):
        m = owner == c
        ch = chunk[m]
        for g in range(N_CHUNKS):
            sel = np.nonzero(m)[0][ch == g]
            per_core[c][g] = sel
            cnt[c, g] = len(sel)
            if len(sel):
                maxdeg[c, g] = np.bincount(dst[sel] - c * NPC).max()
    nb = [
        int(max(np.ceil(cnt[:, g].max() * 1.03 / BLOCK), maxdeg[:, g].max()))
        for g in range(N_CHUNKS)
    ]
    nbtot = sum(nb)

    # packed per-block indices: cols 0:64 src, 64:128 dst, 128:192 code
    idx_all = np.zeros((N_CORES, nbtot, 128, 192), np.int16)
    for c in range(N_CORES):
        b0 = 0
        for g in range(N_CHUNKS):
            sel = per_core[c][g]
            nbg = nb[g]
            n = nbg * BLOCK
            s16 = np.full(n, 0, np.int64)
            d16 = np.full(n, NPCP - 1, np.int64)   # pad -> scratch row
            c16 = np.full(n, 476, np.int64)        # pad -> zero table row
            if len(sel):
                dl = dst[sel] - c * NPC
                order = np.argsort(dl, kind="stable")
                sel = sel[order]
                dl = dl[order]
                deg = np.bincount(dl, minlength=NPC)
                first = np.zeros(len(dl), bool)
                first[0] = True
                first[1:] = dl[1:] != dl[:-1]
                rank = np.arange(len(dl)) - np.repeat(
                    np.nonzero(first)[0], deg[deg > 0]
                )
                start = np.cumsum(deg) - deg
                blk = (start[dl] + rank) % nbg
                bo = np.argsort(blk, kind="stable")
                fills = np.bincount(blk, minlength=nbg)
                assert fills.max() <= BLOCK, (c, g, fills.max())
                pos = np.concatenate(
                    [b * BLOCK + np.arange(f) for b, f in enumerate(fills)]
                )
                s16[pos] = grow[sel[bo]] - g * CHUNK_ROWS
                d16[pos] = dl[bo]
                c16[pos] = code[sel[bo]]
            idx_all[c, b0 : b0 + nbg, :, 0:64] = _wrap16(s16, nbg)
            idx_all[c, b0 : b0 + nbg, :, 64:128] = _wrap16(d16, nbg)
            idx_all[c, b0 : b0 + nbg, :, 128:192] = _wrap16(c16, nbg)
            b0 += nbg

    # molecule one-hot: oh[c, t, n, m] = 1 iff batch of node (c, t*128+n)
    # equals molw0[c] + m, with molw0 the core's static 1024-mol window.
    molw0 = []
    oh = np.zeros((N_CORES, NT, 128, MOLW), np.float32)
    for c in range(N_CORES):
        bl = batch[c * NPC : (c + 1) * NPC]
        w0 = min((bl[0] // 512) * 512, MOLS - MOLW)
        molw0.append(int(w0))
        assert bl[-1] - w0 < MOLW
        tt = np.arange(NPC) // 128
        nn = np.arange(NPC) % 128
        oh[c, tt, nn, bl - w0] = 1.0
    tables = dict(
        cc_cat=cc,
        wu2=np.ascontiguousarray(Wu[80:144]),
        w1=np.asarray(inputs["W1"], np.float32),
        w2=np.asarray(inputs["W2"], np.float32),
        b1=np.asarray(inputs["b1"], np.float32).reshape(128, 1),
        b2=np.full((128, 1), float(np.asarray(inputs["b2"]).reshape(-1)[0]), np.float32),
    )
    per_core_inputs = []
    for c in range(N_CORES):
        m = dict(tables)
        m["idx_all"] = idx_all[c]
        m["oh"] = oh[c]
        per_core_inputs.append(m)
    return per_core_inputs, nb, nbtot, tuple(molw0)


def _build(nb, nbtot, molw0):
    nc = bacc.Bacc(
        "TRN2", target_bir_lowering=False, debug=False, num_devices=N_CORES,
        num_swdge_queues=4,
    )
    t_cc = nc.dram_tensor("cc_cat", [477, 128], F32, kind="ExternalInput")
    t_wu2 = nc.dram_tensor("wu2", [64, 64], F32, kind="ExternalInput")
    t_w1 = nc.dram_tensor("w1", [64, 128], F32, kind="ExternalInput")
    t_w2 = nc.dram_tensor("w2", [128, 1], F32, kind="ExternalInput")
    t_b1 = nc.dram_tensor("b1", [128, 1], F32, kind="ExternalInput")
    t_b2 = nc.dram_tensor("b2", [128, 1], F32, kind="ExternalInput")
    t_idx = nc.dram_tensor("idx_all", [nbtot, 128, 192], I16, kind="ExternalInput")
    t_oh = nc.dram_tensor("oh", [NT, 128, MOLW], F32, kind="ExternalInput")
    t_out = nc.dram_tensor("out", [2048], F32, kind="ExternalOutput")
    DBG = os.environ.get("DMPNN_DEBUG") == "1"
    if DBG:
        t_dagg = nc.dram_tensor("d_agg", [NPCP, MSG], F32, kind="ExternalOutput")
        t_daggw = nc.dram_tensor("d_aggw", [NPCP, MSG], F32, kind="ExternalOutput")
        t_daggwf = nc.dram_tensor("d_aggwf", [NPCP, MSG], F32, kind="ExternalOutput")
        t_dbase = nc.dram_tensor("d_base", [128, TB * MSG], F32, kind="ExternalOutput")
        t_dmsg = nc.dram_tensor("d_msg", [128, TB * MSG], F32, kind="ExternalOutput")
        t_dgath = nc.dram_tensor("d_gath", [128, TB * MSG], F32, kind="ExternalOutput")

    groups = []
    b0 = 0
    for g in range(N_CHUNKS):
        groups.append((b0, nb[g]))
        b0 += nb[g]

    with tile.TileContext(nc) as tc:
        with (
            tc.tile_pool(name="dram", bufs=1, space="DRAM") as dram,
            tc.tile_pool(name="const", bufs=1) as constp,
            tc.tile_pool(name="sb", bufs=int(os.environ.get("DMPNN_SBUFS", "6"))) as sb,
            tc.tile_pool(name="nsb", bufs=2) as nsb,
            tc.tile_pool(name="cmb", bufs=1) as cmbp,
            tc.tile_pool(name="nodes", bufs=2) as nodes,
            tc.tile_pool(name="psum", bufs=2, space="PSUM") as psum,
            tc.tile_pool(name="psum1", bufs=1, space="PSUM") as psum1,
        ):
            accs = []
            for a in range(NACC):
                acc_t = dram.tile([NPCP, MSG], F32, tag=f"acc{a}", name=f"acc{a}")
                accs.append(acc_t)
            agg = dram.tile([NPCP, MSG], F32)            # combined node slice
            aggw_s = dram.tile([NPCP, MSG], F32)
            aggw_f = dram.tile([FULL_ROWS, MSG], F32)
            base_u = dram.tile([nbtot, 128, TB * MSG], F32)
            molg_in = dram.tile([64, MOLW], F32)
            molg_out = dram.tile([N_CORES * 64, MOLW], F32)

            ident = constp.tile([128, 128], F32)
            make_identity(nc, ident[:])
            wu2 = constp.tile([64, 64], F32)
            nc.sync.dma_start(wu2[:], t_wu2[:, :])
            w1 = constp.tile([64, 128], F32)
            nc.sync.dma_start(w1[:], t_w1[:, :])
            w2 = constp.tile([128, 1], F32)
            nc.sync.dma_start(w2[:], t_w2[:, :])
            b1 = constp.tile([128, 1], F32)
            nc.sync.dma_start(b1[:], t_b1[:, :])
            b2v = constp.tile([128, 1], F32)
            nc.sync.dma_start(b2v[:], t_b2[:, :])
            ZC = NPCP * MSG // 256  # 3136
            zeros = constp.tile([128, ZC], F32)
            nc.vector.memset(zeros[:], 0.0)

            def clear(buf, rows):
                flat = buf[:].rearrange("(p t) f -> p (t f)", p=128)
                cols = rows * MSG // 128
                for o in range(0, cols, ZC):
                    w = min(ZC, cols - o)
                    nc.sync.dma_start(flat[:, o : o + w], zeros[:, :w])

            def edge_pass(rnd, dbg=False):
                for a in range(NACC):
                    clear(accs[a], NPCP)
                bi_ = 0
                for g, (gb0, gnb) in enumerate(groups):
                    for b in range(gb0, gb0 + gnb):
                        acc = accs[bi_ % NACC]
                        bi_ += 1
                        msg = sb.tile([128, TB, MSG], F32, tag="msg")
                        if rnd == 0:
                            idxt = sb.tile([128, 128], I16, tag="idxt")
                            nc.sync.dma_start(idxt[:], t_idx[b][:, 64:192])
                            gath = sb.tile([128, TB, 128], F32, tag="gath")
                            nc.gpsimd.dma_gather(
                                gath[:, :, :], t_cc[:, :], idxt[:, 64:128],
                                BLOCK, BLOCK, 128, queue_num=(bi_ % 4) if QMODE else (bi_ % 2),
                            )
                            nc.scalar.activation(
                                msg[:, :, :], gath[:, :, 0:64],
                                mybir.ActivationFunctionType.Relu,
                            )
                            nc.scalar.dma_start(
                                base_u[b].rearrange("p (a b) -> p a b", b=MSG),
                                gath[:, :, 64:128],
                            )
                            nc.gpsimd.dma_scatter_add(
                                acc[:], msg[:, :, :], idxt[:, 0:64],
                                BLOCK, BLOCK, MSG, queue_num=((bi_ + 2) % 4) if QMODE else (2 + bi_ % 2),
                            )
                        else:
                            idxt = sb.tile([128, 128], I16, tag="idxt")
                            nc.sync.dma_start(idxt[:], t_idx[b][:, 0:128])
                            gath = sb.tile([128, TB, MSG], F32, tag="gath")
                            nc.gpsimd.dma_gather(
                                gath[:, :, :],
                                aggw_f[g * CHUNK_ROWS : (g + 1) * CHUNK_ROWS, :],
                                idxt[:, 0:64], BLOCK, BLOCK, MSG,
                                queue_num=(bi_ % 4) if QMODE else (bi_ % 2),
                            )
                            bb = sb.tile([128, TB, MSG], F32, tag="base")
                            nc.scalar.dma_start(
                                bb[:].rearrange("p a b -> p (a b)"), base_u[b]
                            )
                            summ = sb.tile([128, TB * MSG], F32, tag="summ")
                            nc.vector.tensor_tensor(
                                out=summ[:],
                                in0=gath[:].rearrange("p a b -> p (a b)"),
                                in1=bb[:].rearrange("p a b -> p (a b)"),
                                op=mybir.AluOpType.add,
                            )
                            nc.scalar.activation(
                                msg[:].rearrange("p a b -> p (a b)"), summ[:],
                                mybir.ActivationFunctionType.Relu,
                            )
                            scq = ((bi_ + 2) % 4) if QMODE else (2 + bi_ % 2)
                            if dbg and b == 0:
                                nc.sync.dma_start(t_dbase[:, :], bb[:].rearrange("p a b -> p (a b)"))
                                nc.sync.dma_start(t_dmsg[:, :], msg[:].rearrange("p a b -> p (a b)"))
                                nc.sync.dma_start(t_dgath[:, :], gath[:].rearrange("p a b -> p (a b)"))
                            nc.gpsimd.dma_scatter_add(
                                acc[:], msg[:, :, :], idxt[:, 64:128],
                                BLOCK, BLOCK, MSG, queue_num=scq,
                            )

            def combine():
                # agg = sum(accs), tree reduce per chunk
                H = ZC // 2  # 1568
                for o in range(0, 4 * H, H):
                    parts = []
                    for a in range(NACC):
                        pt = cmbp.tile([128, H], F32, tag=f"cmb{a}", name=f"cmb{a}")
                        nc.sync.dma_start(
                            pt[:],
                            accs[a][:].rearrange("(p t) f -> p (t f)", p=128)[
                                :, o : o + H
                            ],
                        )
                        parts.append(pt)
                    lvl = 0
                    while len(parts) > 1:
                        nxt = []
                        for j in range(0, len(parts), 2):
                            sx = cmbp.tile(
                                [128, H], F32, tag=f"cmbs{lvl}_{j}",
                                name=f"cmbs{lvl}_{j}",
                            )
                            nc.vector.tensor_tensor(
                                out=sx[:], in0=parts[j][:], in1=parts[j + 1][:],
                                op=mybir.AluOpType.add,
                            )
                            nxt.append(sx)
                        parts = nxt
                        lvl += 1
                    nc.sync.dma_start(
                        agg[:].rearrange("(p t) f -> p (t f)", p=128)[:, o : o + H],
                        parts[0][:],
                    )

            def node_stage():
                spans = [(q * 512, 512) for q in range(NPCP // 512)]
                if NPCP % 512:
                    spans.append((NPCP - NPCP % 512, NPCP % 512))
                for r0, rn in spans:
                    kt = rn // 128
                    a4 = nodes.tile([128, 4, MSG], F32, tag="a4")
                    nc.sync.dma_start(
                        a4[:, :kt, :],
                        agg[r0 : r0 + rn, :].rearrange("(t p) f -> p t f", p=128),
                    )
                    atp = psum.tile([64, 512], F32, tag="atp", space="PSUM")
                    for k in range(kt):
                        nc.tensor.transpose(
                            atp[:, k * 128 : (k + 1) * 128], a4[:, k, :], ident[:]
                        )
                    ats = nodes.tile([64, 512], F32, tag="ats")
                    nc.vector.tensor_copy(ats[:, : kt * 128], atp[:, : kt * 128])
                    w4 = nodes.tile([128, 4, MSG], F32, tag="w4")
                    for k in range(kt):
                        wp = psum.tile([128, MSG], F32, tag="wp", space="PSUM")
                        nc.tensor.matmul(
                            wp[:], ats[:, k * 128 : (k + 1) * 128], wu2[:],
                            start=True, stop=True,
                        )
                        nc.vector.tensor_copy(w4[:, k, :], wp[:])
                    nc.sync.dma_start(
                        aggw_s[r0 : r0 + rn, :].rearrange("(t p) f -> p t f", p=128),
                        w4[:, :kt, :],
                    )

            def dump(dst_dram, src_dram, rows):
                for t0 in range(0, rows - 511, 512):
                    dt_ = nsb.tile([128, 4, MSG], F32, tag="dump")
                    nc.sync.dma_start(
                        dt_[:, :, :],
                        src_dram[t0 : t0 + 512, :].rearrange("(t p) f -> p t f", p=128),
                    )
                    nc.sync.dma_start(
                        dst_dram[t0 : t0 + 512, :].rearrange("(t p) f -> p t f", p=128),
                        dt_[:, :, :],
                    )

            edge_pass(0)
            for _r in range(ROUNDS):
                combine()
                if DBG and _r == 0:
                    dump(t_dagg, agg, NPCP)
                node_stage()
                if DBG and _r == 0:
                    dump(t_daggw, aggw_s, NPCP)
                nc.gpsimd.collective_compute(
                    "AllGather", mybir.AluOpType.bypass,
                    replica_groups=[list(range(N_CORES))],
                    ins=[aggw_s[:]], outs=[aggw_f[:]],
                )
                if DBG and _r == 0:
                    dump(t_daggwf, aggw_f, NPCP)
                edge_pass(1, dbg=DBG and _r == 0)
            combine()

            # molecules: molT_win[f, m] = sum_t ns_t^T @ oh_t  (PSUM-resident)
            molp = psum1.tile([64, MOLW], F32, tag="molp", space="PSUM")
            for t in range(NT):
                nst = nsb.tile([128, MSG], F32, tag="nst")
                nc.sync.dma_start(nst[:], agg[t * 128 : (t + 1) * 128, :])
                oht = nsb.tile([128, MOLW], F32, tag="oht")
                nc.sync.dma_start(oht[:], t_oh[t])
                for h in range(MOLW // 512):
                    nc.tensor.matmul(
                        molp[:, h * 512 : (h + 1) * 512], nst[:],
                        oht[:, h * 512 : (h + 1) * 512],
                        start=(t == 0), stop=(t == NT - 1),
                    )
            molw_sb = nodes.tile([64, MOLW], F32, tag="molw")
            nc.vector.tensor_copy(molw_sb[:], molp[:])
            nc.sync.dma_start(molg_in[:], molw_sb[:])
            nc.gpsimd.collective_compute(
                "AllGather", mybir.AluOpType.bypass,
                replica_groups=[list(range(N_CORES))],
                ins=[molg_in[:]], outs=[molg_out[:]],
            )
            molT = nodes.tile([64, MOLS], F32, tag="molT")
            nc.vector.memset(molT[:], 0.0)
            for c in range(N_CORES):
                gc = nodes.tile([64, MOLW], F32, tag="molw")
                nc.sync.dma_start(gc[:], molg_out[c * 64 : (c + 1) * 64, :])
                w0 = molw0[c]
                nc.vector.tensor_tensor(
                    out=molT[:, w0 : w0 + MOLW],
                    in0=molT[:, w0 : w0 + MOLW],
                    in1=gc[:],
                    op=mybir.AluOpType.add,
                )

            # readout: hT = relu(W1^T @ molT + b1); out = hT^T @ W2 + b2
            hT = nodes.tile([128, MOLS], F32, tag="hT")
            for q in range(MOLS // 512):
                hp = psum.tile([128, 512], F32, tag="atp", space="PSUM")
                nc.tensor.matmul(
                    hp[:], w1[:], molT[:, q * 512 : (q + 1) * 512],
                    start=True, stop=True,
                )
                nc.scalar.activation(
                    hT[:, q * 512 : (q + 1) * 512], hp[:],
                    mybir.ActivationFunctionType.Relu, bias=b1[:, :1],
                )
            ot = nodes.tile([128, 16], F32, tag="ot")
            for q in range(16):
                op_ = psum.tile([128, 1], F32, tag="wp", space="PSUM")
                nc.tensor.matmul(
                    op_[:], hT[:, q * 128 : (q + 1) * 128], w2[:],
                    start=True, stop=True,
                )
                nc.vector.tensor_copy(ot[:, q : q + 1], op_[:])
            ob = nodes.tile([128, 16], F32, tag="ob")
            nc.vector.tensor_scalar_add(ob[:], ot[:], b2v[:, :1])
            nc.sync.dma_start(t_out[:].rearrange("(t p) -> p t", p=128), ob[:])

    nc.compile()
    return nc


def kernel(**inputs):
    per_core_inputs, nb, nbtot, molw0 = _prep(inputs)
    key = (tuple(nb), molw0)
    if key not in _CACHE:
        _CACHE[key] = _build(nb, nbtot, molw0)
    nc = _CACHE[key]
    res = bass_utils.run_bass_kernel_spmd(
        nc, per_core_inputs, core_ids=list(range(N_CORES))
    )
    return np.asarray(res.results[0]["out"], np.float32)
